# revision 52
# baseline (speedup 1.0000x reference)
"""Trainium2 Bass kernel for LocalNodeAttentionHeadSum (v4).

Computation (per batch b, pixel p=(h,w)):
    q[d,p]   = sum_c x[c,TMID,p] Wq[c,d] + bq[d]
    k[t,d]   = sum_c nodes[t,c] Wk[c,d] + bk[d]
    s[t,p]   = sum_d q[d,p] k[t,d];  alpha = softmax_t(s)
    y[d,p]   = sum_t alpha[t,p] * (sum_c x[c,t,p] Wv[c,d] + bv[d])
    out[c,p] = sum_d y[d,p] Wo[d,c] + bo[c]

Weight-only algebra is folded on the host (inference-style constant
folding; no x-dependent work moves off device):
    kT    = nodes @ Wk + bk                     [T, D]
    Wqk   = Wq @ kT.T                           [C, T]   (scores = x_mid.T Wqk + sb0)
    sb0   = kT @ bq                             [T]
    Wf    = Wv @ Wo   (bf16)                    [C, C]   (value+output proj fused;
                                                          valid because sum_t alpha = 1
                                                          commutes the temporal sum past Wv)
    bo_e  = bv @ Wo + bo (bf16)                 [C]

Device-side per batch: scores via 8 thin fp32 matmuls on the middle
frame (fp32 is required: bf16-rounded score inputs amplify through the
exp to ~5% output error), softmax over T=7 in [T, pix] layout, alpha
broadcast to 128 partitions via indicator matmuls, the alpha-weighted
temporal sum fused per quarter (ACT pre-cast -> DVE mul/tree -> Pool
final add, with the middle-frame term precomputed on Pool), then a
single fused [C->C] bf16 projection accumulated in PSUM with the bias
seeded by a masked matmul, and one merged store per batch.

Sharding: data-parallel over batch B=32 across 8 cores (4 per core).
DMA floor per core ~78us (22.5MB x + 2MB Wf + 3.2MB out at 360GB/s);
the schedule keeps the single DMA resource near-continuously busy:
mid-frames first (scores/softmax unblock early), then the other six
frames stream in uneven stages (2,2,2,1,1 chunks) so the tail stage is
small and drains fast.
"""

import sys

for _p in ("/opt/trn_rl_repo",):
    if _p not in sys.path:
        sys.path.insert(0, _p)

from contextlib import ExitStack

import numpy as np

import concourse.bass as bass
import concourse.tile as tile
from concourse import bacc, mybir, bass_isa
from concourse.bass_utils import run_bass_kernel_spmd

F32 = mybir.dt.float32
BF16 = mybir.dt.bfloat16

# Problem shapes (hardcoded per contract)
B, C, T, H, W = 32, 1024, 7, 14, 14
D = 512
NCORES = 8
BL = B // NCORES          # 4 batches per core
HWF = H * W               # 196
THW = T * HWF             # 1372
CC = C // 128             # 8 chunks over channels
TMID = T // 2             # 3 (middle frame)
F2 = 2 * HWF              # 392: the two batches of a pair along free axis
REST = 6 * HWF            # 1176: the six non-middle frames of one chunk
HALF = 3 * HWF            # 588

Exp = mybir.ActivationFunctionType.Exp

# rest stages: (first chunk, last chunk) — uneven so the tail is small
QCH = [(0, 2), (2, 4), (4, 6), (6, 7), (7, 8)]
NQ = len(QCH)

# engine for each alpha-broadcast psum->sbuf copy, per t (per batch).
# GPSIMD cannot touch PSUM on real HW, so only act/dve are legal here.
AB_ENG = {t: "act" for t in range(T)}
# engines for the 4 output-tile psum->sbuf copies, per batch (act/dve)
OB_ENG = {0: ["act"] * 4, 1: ["act"] * 4,
          2: ["act"] * 4, 3: ["act", "dve", "act", "dve"]}
# x-cast engine per (stage, local batch): ACT by default, Pool for a few
# early stages to balance load (SBUF->SBUF copies are legal on GPSIMD),
# and 'dve' = no cast at all (DVE muls read fp32 directly) for the tail
# stages so the last chunks skip the ACT hop entirely.
# keys may be (q, l) or (pr, q, l); 'dvec' = bf16 cast on DVE,
# 'skip' = no cast (full-Pool tree reads fp32 directly)
CAST_ENG = {(0, 1): "pool", (1, 1): "pool", (0, 0, 1): "dvec", (0, 1, 1): "dvec",
            (1, 3, 1): "pool", (1, 4, 1): "pool", (1, 1, 1): "skip"}
# stages whose whole mul/tree/final-add chain runs on GpSimd
POOL_TREE = {(1, 1, 1)}


def build_program():
    nc = bacc.Bacc("TRN2", target_bir_lowering=False, debug=False)

    x_d = nc.dram_tensor("x_window", [BL, C, T, H, W], F32, kind="ExternalInput").ap()
    wf_d = nc.dram_tensor("Wf", [C, C], BF16, kind="ExternalInput").ap()
    wqk_d = nc.dram_tensor("Wqk", [C, T], F32, kind="ExternalInput").ap()
    sb0_d = nc.dram_tensor("sb0", [1, T], F32, kind="ExternalInput").ap()
    bo_d = nc.dram_tensor("bo_e", [2, D], BF16, kind="ExternalInput").ap()
    out_d = nc.dram_tensor("out", [BL, C, 1, H, W], F32, kind="ExternalOutput").ap()

    x_r = x_d.rearrange("b (cc p) t h w -> b p cc (t h w)", p=128)
    out_r = out_d.rearrange("b (cc p) o h w -> b p cc (o h w)", p=128)
    wf_r = wf_d.rearrange("(cc p) c2 -> p cc c2", p=128)
    wqk_r = wqk_d.rearrange("(cc p) t -> p cc t", p=128)

    with tile.TileContext(nc) as tc, ExitStack() as ctx:
        cpool = ctx.enter_context(tc.tile_pool(name="const", bufs=1))
        midpool = ctx.enter_context(tc.tile_pool(name="mid", bufs=2))
        restpool = ctx.enter_context(tc.tile_pool(name="rest", bufs=3))
        rest1pool = ctx.enter_context(tc.tile_pool(name="rest1", bufs=2))
        abpool = ctx.enter_context(tc.tile_pool(name="ab", bufs=2))
        xwpool = ctx.enter_context(tc.tile_pool(name="xw", bufs=2))
        xcpool = ctx.enter_context(tc.tile_pool(name="xc", bufs=4))
        tmpool = ctx.enter_context(tc.tile_pool(name="tm", bufs=2))
        tmapool = ctx.enter_context(tc.tile_pool(name="tma", bufs=2))
        smpool = ctx.enter_context(tc.tile_pool(name="sm", bufs=2))
        obpool = ctx.enter_context(tc.tile_pool(name="ob", bufs=2))
        psp = ctx.enter_context(tc.tile_pool(name="ps", bufs=1, space="PSUM"))

        # ---- constants (SWDGE queue; SP stays clear for the x stream) ----
        # PE warmup: the p-state ramp needs ~3us of continuous matmul
        # activity to reach 2.4GHz; idle drops it to 0.65/1.2GHz. Dummy
        # matmuls bridge the gap until the first (fp32, latency-critical)
        # score matmuls so they run at full clock.
        warmrhs = cpool.tile([T, F2], BF16)
        nc.gpsimd.memset(warmrhs[:], 0.0)
        wones7c = cpool.tile([T, 1], BF16)
        nc.gpsimd.memset(wones7c[:], 1.0)
        wqk_sb = cpool.tile([128, CC * T], F32)
        nc.gpsimd.dma_start(
            wqk_sb[:].rearrange("p (cc t) -> p cc t", t=T), wqk_r
        )
        sb0_sb = cpool.tile([1, T], F32)
        nc.gpsimd.dma_start(sb0_sb[:], sb0_d)
        bo_sb = cpool.tile([2, D], BF16)
        nc.gpsimd.dma_start(bo_sb[:], bo_d)
        import ml_dtypes

        e_np = np.zeros((T, T * 128), dtype=ml_dtypes.bfloat16)
        for t in range(T):
            e_np[t, t * 128 : (t + 1) * 128] = 1.0
        e_dram = nc.inline_tensor(e_np, name="e_ind")
        e_all = cpool.tile([T, T * 128], BF16)
        nc.gpsimd.dma_start(e_all[:], e_dram.ap())
        Es = [e_all[:, t * 128 : (t + 1) * 128] for t in range(T)]

        # [2, 392] half-selector: row0 hits cols 0:196, row1 cols 196:392 —
        # lets one matmul seed different cc' biases into each tile half.
        m_np = np.zeros((2, F2), dtype=ml_dtypes.bfloat16)
        m_np[0, 0:HWF] = 1.0
        m_np[1, HWF:F2] = 1.0
        m_dram = nc.inline_tensor(m_np, name="halfmask")
        msk = cpool.tile([2, F2], BF16)
        nc.gpsimd.dma_start(msk[:], m_dram.ap())

        ones196 = cpool.tile([1, HWF], F32)
        nc.gpsimd.memset(ones196[:], 1.0)
        ones7c = cpool.tile([T, 1], BF16)
        nc.gpsimd.memset(ones7c[:], 1.0)

        wf_sb = cpool.tile([128, CC * C], BF16)

        state = [dict() for _ in range(2)]  # per pair

        # Softmax-era psum tiles rotate through the same 4 "pk" slots the
        # odd-batch projections use later (temporally disjoint), keeping
        # total PSUM at 8 banks: pj0-3 + pk0-3.
        _rot = [0]

        _rotj = [0]

        def psum_sm(shape, fam="pk"):
            r = _rot if fam == "pk" else _rotj
            t = psp.tile(shape, F32, tag=f"{fam}{r[0]}", bufs=1, name=f"smps{fam}{r[0]}")
            r[0] = (r[0] + 1) % 4
            return t

        # ---- stage emitters -------------------------------------------
        def emit_mid(pr):
            mid = midpool.tile([128, 2 * CC * HWF], F32, tag="mid")
            for l in range(2):
                b = 2 * pr + l
                nc.sync.dma_start(
                    mid[:, l * CC * HWF : (l + 1) * CC * HWF].rearrange(
                        "p (cc f) -> p cc f", f=HWF
                    ),
                    x_r[b][:, :, TMID * HWF : (TMID + 1) * HWF],
                )
            state[pr]["mid"] = mid

        def emit_wf():
            nc.sync.dma_start(
                wf_sb[:].rearrange("p (cc c2) -> p cc c2", c2=C), wf_r
            )

        def emit_rest(pr, q):
            c0, c1 = QCH[q]
            n = c1 - c0
            pool = restpool if n == 2 else rest1pool
            rq = pool.tile([128, 2 * n * REST], F32, tag=f"rest{n}", name="rq")
            for l in range(2):
                b = 2 * pr + l
                dst = rq[:, l * n * REST : (l + 1) * n * REST].rearrange(
                    "p (cc s) -> p cc s", s=REST
                )
                nc.sync.dma_start(
                    dst[:, :, 0:HALF],
                    x_r[b][:, c0:c1, 0:HALF],
                )
                nc.sync.dma_start(
                    dst[:, :, HALF:REST],
                    x_r[b][:, c0:c1, (TMID + 1) * HWF : THW],
                )
            state[pr][f"rq{q}"] = rq

        def emit_scores(pr, l):
            """Per-batch fp32 score matmuls into a psum tile."""
            st = state[pr]
            mid = st["mid"]
            if l == 0:
                st["ab"] = abpool.tile([128, T * F2], BF16, tag="ab", name="ab")
                st["xw"] = xwpool.tile([128, CC * F2], BF16, tag="xw", name="xw")
                st["tma"] = tmapool.tile(
                    [128, CC * F2], BF16, tag="tma", name="tma"
                )
            sp = psum_sm([T, HWF], "pj" if pr == 1 else "pk")
            for cc in range(CC):
                nc.tensor.matmul(
                    sp[:],
                    wqk_sb[:, cc * T : (cc + 1) * T],
                    mid[:, (l * CC + cc) * HWF : (l * CC + cc + 1) * HWF],
                    start=(cc == 0),
                    stop=False,
                )
            nc.tensor.matmul(sp[:], sb0_sb[:], ones196[:], start=False, stop=True)
            state[pr][f"sp{l}"] = sp

        def emit_sm_a(pr, l):
            sp = state[pr][f"sp{l}"]
            s_sb = smpool.tile([T, HWF], F32, tag="ssb", bufs=1)
            nc.scalar.copy(s_sb[:], sp[:])
            mx = smpool.tile([T, HWF], F32, tag="mx", bufs=1)
            nc.gpsimd.partition_all_reduce(
                mx[:], s_sb[:], channels=T, reduce_op=bass_isa.ReduceOp.max
            )
            sm = smpool.tile([T, HWF], F32, tag="smx", bufs=1)
            nc.vector.tensor_sub(sm[:], s_sb[:], mx[:])
            e_sb = smpool.tile([T, HWF], BF16, tag="e")
            nc.scalar.activation(e_sb[:], sm[:], Exp, bias=0.0, scale=1.0)
            state[pr][f"e{l}"] = e_sb

        def emit_sm_b(pr, l):
            e_sb = state[pr][f"e{l}"]
            zp = psum_sm([1, HWF], "pj" if pr == 1 else "pk")
            nc.tensor.matmul(zp[:], ones7c[:], e_sb[:], start=True, stop=True)
            rz = smpool.tile([1, HWF], F32, tag="rz", bufs=1)
            nc.vector.reciprocal_approx_fast(rz[:], zp[:])
            rb = smpool.tile([T, HWF], F32, tag="rb", bufs=1)
            nc.gpsimd.partition_broadcast(rb[:], rz[:])
            aT = smpool.tile([T, HWF], BF16, tag="aT")
            nc.vector.tensor_mul(aT[:], e_sb[:], rb[:])
            state[pr][f"aT{l}"] = aT

        # alpha rows live in ab in PERMUTED order: position 0 holds t=TMID
        # (the Pool tma product needs it first), positions 1..6 hold
        # t = 0,1,2,4,5,6 — so the two wsum halves see contiguous rows.
        ABPOS = [TMID, 0, 1, 2, 4, 5, 6]

        def emit_bc(pr, l):
            """Broadcast alpha to 128 partitions: t=TMID alone, then three
            adjacent position-pairs per psum tile (one bank each)."""
            st = state[pr]
            ab, aT = st["ab"], st[f"aT{l}"]
            lsl = slice(l * HWF, (l + 1) * HWF)
            fam = "pk"
            abp = psum_sm([128, HWF], fam)
            nc.tensor.matmul(abp[:], Es[TMID], aT[:], start=True, stop=True)
            nc.scalar.copy(ab[:, l * HWF : l * HWF + HWF], abp[:])
            for pi, eng in ((1, "act"), (3, "dve"), (5, "act")):
                pp = psum_sm([128, F2], fam)
                for k in range(2):
                    nc.tensor.matmul(
                        pp[:, k * HWF : (k + 1) * HWF],
                        Es[ABPOS[pi + k]],
                        aT[:],
                        start=True,
                        stop=True,
                    )
                dst = ab[:].rearrange("p (t f) -> p t f", t=T)[
                    :, pi : pi + 2, lsl
                ]
                src = pp[:].rearrange("p (t f) -> p t f", t=2)
                if eng == "act":
                    nc.scalar.copy(dst, src)
                else:
                    nc.vector.tensor_copy(dst, src)

        def emit_tma(pr, l):
            # mid-frame contribution for all 8 chunks in one Pool op
            st = state[pr]
            ab, tma, mid = st["ab"], st["tma"], st["mid"]
            lsl = slice(l * HWF, (l + 1) * HWF)
            tmav = tma[:].rearrange("p (cc bf) -> p cc bf", cc=CC)[:, :, lsl]
            nc.gpsimd.tensor_mul(
                tmav,
                mid[:, l * CC * HWF : (l + 1) * CC * HWF].rearrange(
                    "p (cc f) -> p cc f", f=HWF
                ),
                ab[:, l * HWF : l * HWF + HWF].unsqueeze(1).broadcast_to(
                    (128, CC, HWF)
                ),
            )

        def emit_cast(pr, q, l):
            """Stage x cast to bf16 (or mark for direct-fp32 DVE reads)."""
            st = state[pr]
            rq = st[f"rq{q}"]
            c0, c1 = QCH[q]
            n = c1 - c0
            nr = n * REST
            base = l * nr
            kind = CAST_ENG.get((pr, q, l), CAST_ENG.get((q, l), "act"))
            if kind in ("dve", "skip"):
                st[f"src{q}_{l}"] = rq[:, base : base + nr]
                return
            xc = xcpool.tile([128, nr], BF16, tag="xc", name="xc")
            if kind == "pool":
                nc.gpsimd.tensor_copy(xc[:], rq[:, base : base + nr])
            elif kind == "dvec":
                nc.vector.tensor_copy(xc[:], rq[:, base : base + nr])
            else:
                nc.scalar.copy(xc[:], rq[:, base : base + nr])
            st[f"src{q}_{l}"] = xc[:]

        def emit_tree(pr, q, l):
            """DVE mul + add tree, Pool final add (+ precomputed tma)."""
            st = state[pr]
            ab, xw, tma = st["ab"], st["xw"], st["tma"]
            src = st[f"src{q}_{l}"]
            c0, c1 = QCH[q]
            n = c1 - c0
            nr = n * REST
            srcv = src.rearrange("p (cc s) -> p cc s", cc=n)
            ab7 = ab[:].rearrange("p (t f) -> p t f", t=T)
            lsl = slice(l * HWF, (l + 1) * HWF)
            bc = lambda a: a.unsqueeze(1).broadcast_to((128, n, TMID, HWF))
            eng = nc.gpsimd if (pr, q, l) in POOL_TREE else nc.vector
            tm = tmpool.tile([128, nr], BF16, tag="tm", name="tm")
            eng.tensor_mul(
                tm[:, 0 : nr // 2].rearrange("p (cc tt f) -> p cc tt f", cc=n, f=HWF),
                srcv[:, :, 0:HALF].rearrange("p cc (tt f) -> p cc tt f", f=HWF),
                bc(ab7[:, 1:4, lsl]),
            )
            eng.tensor_mul(
                tm[:, nr // 2 : nr].rearrange("p (cc tt f) -> p cc tt f", cc=n, f=HWF),
                srcv[:, :, HALF:REST].rearrange("p cc (tt f) -> p cc tt f", f=HWF),
                bc(ab7[:, 4:7, lsl]),
            )
            s1 = tmpool.tile([128, nr // 2], BF16, tag="s1", name="s1")
            eng.tensor_add(s1[:], tm[:, 0 : nr // 2], tm[:, nr // 2 : nr])
            s1v = s1[:].rearrange("p (cc s) -> p cc s", cc=n)
            s2 = tmpool.tile([128, n * HWF], BF16, tag="s2", name="s2")
            s2v = s2[:].rearrange("p (cc f) -> p cc f", f=HWF)
            eng.tensor_add(s2v, s1v[:, :, 0:HWF], s1v[:, :, HWF : 2 * HWF])
            s3 = tmpool.tile([128, n * HWF], BF16, tag="s3", name="s3")
            s3v = s3[:].rearrange("p (cc f) -> p cc f", f=HWF)
            eng.tensor_add(s3v, s2v, s1v[:, :, 2 * HWF : HALF])
            xwv = xw[:].rearrange("p (cc bf) -> p cc bf", cc=CC)[:, c0:c1, lsl]
            tmav = tma[:].rearrange("p (cc bf) -> p cc bf", cc=CC)[:, c0:c1, lsl]
            if (pr, q, l) in POOL_TREE:
                nc.gpsimd.tensor_add(xwv, s3v, tmav)
            elif q >= 3:
                nc.vector.tensor_add(xwv, s3v, tmav)
            else:
                nc.gpsimd.tensor_add(xwv, s3v, tmav)

        def emit_proj_bias(role):
            """Allocate the 4 psum tiles for a batch and seed cc' biases
            with one full-width masked matmul each (one psum group/bank)."""
            ptiles = []
            for jj in range(4):
                pt = psp.tile(
                    [128, F2], F32, tag=f"{role}{jj}", bufs=1, name=f"pt{role}{jj}"
                )
                nc.tensor.matmul(
                    pt[:],
                    bo_sb[:, jj * 128 : (jj + 1) * 128],
                    msk[:],
                    start=True,
                    stop=False,
                )
                ptiles.append(pt)
            return ptiles

        def emit_proj_chunk(ptiles, pr, l, cc, stop):
            xw = state[pr]["xw"]
            rhs = xw[:, cc * F2 + l * HWF : cc * F2 + (l + 1) * HWF]
            for jj in range(4):
                for k in range(2):
                    ccp = 2 * jj + k
                    nc.tensor.matmul(
                        ptiles[jj][:, k * HWF : (k + 1) * HWF],
                        wf_sb[:, cc * C + ccp * 128 : cc * C + (ccp + 1) * 128],
                        rhs,
                        start=False,
                        stop=(stop and k == 1),
                    )

        def emit_ob(ptiles, b):
            ob = obpool.tile([128, CC * HWF], F32, tag="ob", name="ob")
            for jj in range(4):
                dst = ob[:, jj * F2 : (jj + 1) * F2]
                if OB_ENG[b][jj] == "act":
                    nc.scalar.copy(dst, ptiles[jj][:])
                else:
                    nc.vector.tensor_copy(dst, ptiles[jj][:])
            return ob

        def emit_st(ob, b):
            # Stores go on the SP queue, emitted AFTER every load DMA: the
            # HWDGE completion sems are assigned round-robin over the final
            # instruction order, and a lane's next DMA waits for its
            # predecessor — a store that waits on late compute must never
            # precede a load on its sem lane. Same-queue FIFO pins the order.
            nc.sync.dma_start(
                out_r[b], ob[:].rearrange("p (cc f) -> p cc f", f=HWF)
            )

        # ---- emission schedule ----------------------------------------
        warmp = psum_sm([1, F2])
        for _ in range(16):
            nc.tensor.matmul(warmp[:], wones7c[:], warmrhs[:], start=True, stop=True)
        emit_mid(0)
        emit_mid(1)
        emit_scores(0, 0)
        emit_scores(0, 1)
        emit_sm_a(0, 0)
        emit_sm_a(0, 1)
        emit_sm_b(0, 0)
        emit_sm_b(0, 1)
        emit_bc(0, 0)
        emit_tma(0, 0)
        emit_bc(0, 1)
        emit_tma(0, 1)
        emit_rest(0, 0)
        emit_wf()
        emit_rest(0, 1)
        emit_cast(0, 0, 0)
        emit_cast(0, 0, 1)
        emit_cast(0, 1, 0)
        emit_cast(0, 1, 1)
        emit_scores(1, 0)
        emit_scores(1, 1)
        emit_sm_a(1, 0)
        emit_sm_a(1, 1)
        pj_b0 = emit_proj_bias("pj")
        emit_tree(0, 0, 0)
        emit_tree(0, 0, 1)
        emit_proj_chunk(pj_b0, 0, 0, 0, stop=False)
        emit_proj_chunk(pj_b0, 0, 0, 1, stop=False)
        emit_sm_b(1, 0)
        emit_sm_b(1, 1)
        emit_rest(0, 2)
        emit_cast(0, 2, 0)
        emit_cast(0, 2, 1)
        emit_tree(0, 1, 0)
        emit_tree(0, 1, 1)
        emit_proj_chunk(pj_b0, 0, 0, 2, stop=False)
        emit_proj_chunk(pj_b0, 0, 0, 3, stop=False)
        emit_bc(1, 0)
        emit_tma(1, 0)
        emit_bc(1, 1)
        emit_tma(1, 1)
        emit_rest(0, 3)
        emit_rest(0, 4)
        emit_cast(0, 3, 0)
        emit_cast(0, 3, 1)
        emit_cast(0, 4, 0)
        emit_cast(0, 4, 1)
        emit_tree(0, 2, 0)
        emit_tree(0, 2, 1)
        emit_proj_chunk(pj_b0, 0, 0, 4, stop=False)
        emit_proj_chunk(pj_b0, 0, 0, 5, stop=False)
        emit_rest(1, 0)
        emit_cast(1, 0, 0)
        emit_cast(1, 0, 1)
        emit_tree(0, 3, 0)
        emit_tree(0, 3, 1)
        emit_tree(0, 4, 0)
        emit_tree(0, 4, 1)
        emit_proj_chunk(pj_b0, 0, 0, 6, stop=False)
        emit_proj_chunk(pj_b0, 0, 0, 7, stop=True)
        emit_rest(1, 1)
        emit_cast(1, 1, 0)
        emit_cast(1, 1, 1)
        emit_tree(1, 0, 0)
        emit_tree(1, 0, 1)
        ob0 = emit_ob(pj_b0, 0)
        pk_b1 = emit_proj_bias("pk")
        for cc in range(CC):
            emit_proj_chunk(pk_b1, 0, 1, cc, stop=(cc == CC - 1))
        ob1 = emit_ob(pk_b1, 1)
        emit_rest(1, 2)
        emit_cast(1, 2, 0)
        emit_cast(1, 2, 1)
        pj_b2 = emit_proj_bias("pj")
        emit_tree(1, 1, 0)
        emit_tree(1, 1, 1)
        for cc in range(2):
            emit_proj_chunk(pj_b2, 1, 0, cc, stop=False)
        pk_b3 = emit_proj_bias("pk")
        for cc in range(2):
            emit_proj_chunk(pk_b3, 1, 1, cc, stop=False)
        emit_rest(1, 3)
        emit_rest(1, 4)
        emit_cast(1, 3, 0)
        emit_cast(1, 3, 1)
        emit_cast(1, 4, 0)
        emit_cast(1, 4, 1)
        emit_st(ob0, 0)
        emit_st(ob1, 1)
        emit_tree(1, 2, 0)
        emit_tree(1, 2, 1)
        for cc in range(2, 4):
            emit_proj_chunk(pj_b2, 1, 0, cc, stop=False)
            emit_proj_chunk(pk_b3, 1, 1, cc, stop=False)
        for cc in range(4, 6):
            emit_proj_chunk(pj_b2, 1, 0, cc, stop=False)
            emit_proj_chunk(pk_b3, 1, 1, cc, stop=False)
        emit_tree(1, 3, 0)
        emit_tree(1, 4, 0)
        emit_proj_chunk(pj_b2, 1, 0, 6, stop=False)
        emit_proj_chunk(pj_b2, 1, 0, 7, stop=True)
        ob2 = emit_ob(pj_b2, 2)
        emit_st(ob2, 2)
        emit_tree(1, 3, 1)
        emit_tree(1, 4, 1)
        emit_proj_chunk(pk_b3, 1, 1, 6, stop=False)
        emit_proj_chunk(pk_b3, 1, 1, 7, stop=True)
        ob3 = emit_ob(pk_b3, 3)
        emit_st(ob3, 3)

    nc.compile()
    return nc


_PROG = None


def _get_prog():
    global _PROG
    if _PROG is None:
        _PROG = build_program()
    return _PROG


def _shard_inputs(inputs):
    import ml_dtypes

    f = lambda k: np.asarray(inputs[k], dtype=np.float64)
    x = np.ascontiguousarray(np.asarray(inputs["x_window"], dtype=np.float32))
    nodes, Wq, bq, Wk, bk = f("nodes"), f("Wq"), f("bq"), f("Wk"), f("bk")
    Wv, bv, Wo, bo = f("Wv"), f("bv"), f("Wo"), f("bo")
    kT = nodes @ Wk + bk                                   # [T, D]
    shared = {
        "Wqk": np.ascontiguousarray((Wq @ kT.T).astype(np.float32)),
        "sb0": np.ascontiguousarray((kT @ bq).astype(np.float32).reshape(1, T)),
        "Wf": np.ascontiguousarray((Wv @ Wo).astype(ml_dtypes.bfloat16)),
        "bo_e": np.ascontiguousarray(
            (bv @ Wo + bo)
            .astype(ml_dtypes.bfloat16)
            .reshape(CC, 128)
            .reshape(CC // 2, 2, 128)
            .transpose(1, 0, 2)
            .reshape(2, D)
        ),
    }
    in_maps = []
    for i in range(NCORES):
        m = dict(shared)
        m["x_window"] = np.ascontiguousarray(x[i * BL : (i + 1) * BL])
        in_maps.append(m)
    return in_maps


def kernel(**inputs):
    nc = _get_prog()
    in_maps = _shard_inputs(inputs)
    res = run_bass_kernel_spmd(nc, in_maps, core_ids=list(range(NCORES)))
    return np.concatenate([res.results[i]["out"] for i in range(NCORES)], axis=0)
